# revision 38
# baseline (speedup 1.0000x reference)
"""BayesianLinear Trainium2 kernel, 8-core SPMD (data-parallel over batch).

Per-core computation (4 samples each):
    w_b = weight_mean + noise_b * exp(0.5 * weight_logvar)   (B,O,I)
    out_b = x_b @ w_b^T + bias                               (B,L,O)

Design (per core) — DMA-roofline oriented (~40 MB/core HBM traffic,
~112 us at the 358 GB/s per-core HBM limit; measured 141-155 us wall):
  - All matmul operands are bf16 (tolerance 2e-2; this path lands 2.4e-3):
    noise/x/mean are cast f32->bf16 *during* the DMA load (SWDGE cast),
    std = exp(0.5*logvar) is produced in bf16 by ACT directly (logvar
    stays f32 for exp accuracy).
  - bf16 PE transposes run at ~56 ns/block warm (FWL weight loads);
    grouped 8-deep per PSUM bank so each evac/add is one [128,8,128] op.
  - GEMM: per l-tile one 2-bank PSUM tile; each xT stationary is shared
    by two N=512 matmuls (both output halves), halving LDWEIGHTS (216 ns
    vs 259 ns per matmul).  Bias is added during the PSUM->SBUF evac
    (DVE add against a resident broadcast block) so the PE runs pure
    N=512 matmuls.
  - Software pipelining: sample b+1's transpose groups are interleaved
    between sample b's GEMM l-tiles (no >2us PE stretch without real
    matmuls, keeping the HAM clock gate at 2.4 GHz).  Samples 0 and 3
    run their GEMM half-by-half: sample 0's starts once chunks 0,1
    (~8 MB) have landed; sample 3's second half alone waits for the
    final noise bytes, shortening the tail.
  - Load order is tuned (identity mask before SWDGE descriptor
    emissions; 2 MB noise chunk-pairs; logvar on the HWDGE ring) so the
    SDMA engines stream at line rate from ~8 us (fixed preamble) on.
"""
import numpy as np

SAMPLES = 4           # batch samples per core
N_CORES = 8
B, L, I, O = 32, 512, 1024, 1024
KT = I // 128         # 8 k-tiles (contraction)
OT = O // 128         # 8 o-blocks
LT = L // 128         # 4 l-tiles
NCH = 4               # noise chunks per sample (2 o-blocks each)

_cache = {}


def _split_multi_waits(nc, mybir):
    """This walrus build allows at most one sync-wait per instruction; move
    extra waits onto preceding single-wait NOPs on the same engine.  Safe
    because kernel semaphores are monotonic between resets, so waiting
    sequentially is equivalent to waiting on the conjunction."""
    for fn in nc.m.functions:
        for bb in fn.blocks:
            insts = bb.instructions
            changed = False
            new_list = []
            for inst in insts:
                si = inst.sync_info
                if si is not None and si.on_wait and len(si.on_wait) > 1:
                    waits = list(si.on_wait)
                    for j, w in enumerate(waits[:-1]):
                        nop = mybir.InstNoOp(name=f"{inst.name}-w{j}", ins=[], outs=[])
                        nop.engine = inst.engine
                        nop.sync_info = mybir.SyncInfo(on_wait=[w], on_update=[])
                        new_list.append(nop)
                    inst.sync_info = mybir.SyncInfo(
                        on_wait=[waits[-1]], on_update=list(si.on_update or []))
                    changed = True
                new_list.append(inst)
            if changed:
                bb.instructions = new_list


def build_nc(use_f32r=True):
    from contextlib import ExitStack
    from concourse import bass, mybir, tile, masks

    F32 = mybir.dt.float32
    BF16 = mybir.dt.bfloat16
    Exp = mybir.ActivationFunctionType.Exp
    Copy = mybir.ActivationFunctionType.Copy
    mult = mybir.AluOpType.mult
    add = mybir.AluOpType.add

    nc = bass.Bass()
    x_d = nc.declare_dram_parameter("x", [SAMPLES, L, I], F32, isOutput=False)
    nz_d = nc.declare_dram_parameter("noise", [SAMPLES, O, I], F32, isOutput=False)
    wm_d = nc.declare_dram_parameter("weight_mean", [O, I], F32, isOutput=False)
    wl_d = nc.declare_dram_parameter("weight_logvar", [O, I], F32, isOutput=False)
    b_d = nc.declare_dram_parameter("bias", [O], F32, isOutput=False)
    out_d = nc.declare_dram_parameter("out", [SAMPLES, L, O], F32, isOutput=True)

    with tile.TileContext(nc) as tc, ExitStack() as ctx:
        resident = ctx.enter_context(tc.tile_pool(name="resident", bufs=1))
        lv_pool = ctx.enter_context(tc.tile_pool(name="lv", bufs=1))
        mn_pool = ctx.enter_context(tc.tile_pool(name="mn", bufs=2))
        nz_pool = ctx.enter_context(tc.tile_pool(name="nz", bufs=2))
        sc_pool = ctx.enter_context(tc.tile_pool(name="scn", bufs=2))
        xnat_pool = ctx.enter_context(tc.tile_pool(name="xnat", bufs=1))
        xT_pool = ctx.enter_context(tc.tile_pool(name="xT", bufs=2))
        wT_pool = ctx.enter_context(tc.tile_pool(name="wT", bufs=2))
        out_pool = ctx.enter_context(tc.tile_pool(name="outp", bufs=3))
        psum_mm = ctx.enter_context(tc.tile_pool(name="psum_mm", bufs=2, space="PSUM"))
        psum_nt = ctx.enter_context(tc.tile_pool(name="psum_nt", bufs=2, space="PSUM"))
        psum_xt = ctx.enter_context(tc.tile_pool(name="psum_xt", bufs=2, space="PSUM"))

        # ---------------- residents ----------------
        std_b = resident.tile([128, OT, I], BF16, tag="std")     # exp(.5 lv), natural
        meanT = resident.tile([128, KT, O], BF16, tag="meanT")   # mean^T
        ident_b = resident.tile([128, 128], BF16, tag="ident_b")
        ones_b = resident.tile([1, 128], BF16, tag="ones_b")
        bias_f = resident.tile([1, O], F32, tag="bias_f")
        bias_b = resident.tile([1, O], BF16, tag="bias_b")
        bias_blk = resident.tile([128, O], F32, tag="bias_blk")  # bias bcast to rows

        # identity first: every transpose depends on it, and it must beat the
        # SWDGE descriptor emissions into the GpSimd queue.
        masks.make_identity(nc, ident_b[:])

        # ---------------- earliest DMA issues ----------------
        # SWDGE (gpsimd) order matters: first chunk's mean+noise, then x,
        # then the rest, matching the prologue's consumption order.
        nc.sync.dma_start(bias_f[:], b_d[:].rearrange("(a o) -> a o", a=1))
        mn_tiles, nz_tiles, lv_tiles = {}, {}, {}

        def emit_lv_load(j):
            lv_tiles[j] = lv_pool.tile([128, 2, I], F32, tag="lv", name=f"lv{j}")
            nc.sync.dma_start(
                lv_tiles[j][:],
                wl_d[256 * j:256 * (j + 1), :].rearrange("(q p) i -> p q i", p=128))

        def emit_mn_load(j):
            mn_tiles[j] = mn_pool.tile([128, 2, I], BF16, tag="mn", name=f"mn{j}")
            nc.gpsimd.dma_start(
                mn_tiles[j][:],
                wm_d[256 * j:256 * (j + 1), :].rearrange("(q p) i -> p q i", p=128))

        def emit_nz_load(b, c):
            """noise chunk c of sample b as its own 1 MB SWDGE op: the 2.8us
            arrival quantum stays under the 3.4us HAM window, so DMA-paced
            waits between chunks don't re-throttle the PE clock."""
            if b not in nz_tiles:
                nz_tiles[b] = nz_pool.tile([128, OT, I], BF16, tag="nz",
                                           name=f"nz{b}")
            nc.gpsimd.dma_start(
                nz_tiles[b][:, 2 * c:2 * (c + 1), :],
                nz_d[b, 256 * c:256 * (c + 1), :]
                .rearrange("(q p) i -> p q i", p=128))

        def emit_x_load(b):
            x_tiles[b] = xnat_pool.tile([128, LT, I], BF16, tag="xnat", name=f"xn{b}")
            nc.gpsimd.dma_start(
                x_tiles[b][:], x_d[b].rearrange("(m p) i -> p m i", p=128))

        x_tiles = {}
        emit_lv_load(0), emit_lv_load(1)
        emit_mn_load(0), emit_nz_load(0, 0), emit_x_load(0)
        emit_mn_load(1), emit_nz_load(0, 1)
        emit_lv_load(2), emit_lv_load(3)
        emit_mn_load(2), emit_nz_load(0, 2)
        emit_mn_load(3), emit_nz_load(0, 3)

        # ---------------- setup compute ----------------
        nc.vector.memset(ones_b[:], 1.0)
        nc.vector.tensor_copy(bias_b[:], bias_f[:])


        # ---------------- emitters ----------------
        def emit_mean_group(j, q):
            """transpose mean slab j, column-half q -> meanT o-block 2j+q."""
            ob = 2 * j + q
            mn = mn_tiles[j] if q == 0 else mn_tiles.pop(j)
            pt = psum_nt.tile([128, KT, 128], BF16, tag="pnt")
            for k in range(KT):
                nc.tensor.matmul(pt[:, k, :], mn[:, q, 128 * k:128 * (k + 1)],
                                 ident_b[:], is_transpose=True, start=True, stop=True)
            nc.scalar.activation(meanT[:, :, 128 * ob:128 * (ob + 1)], pt[:], Copy)

        def emit_exp(j):
            nc.scalar.activation(std_b[:, 2 * j:2 * (j + 1), :], lv_tiles.pop(j)[:],
                                 Exp, bias=0.0, scale=0.5)

        sc_tiles = {}

        def emit_scale(b, c):
            """sc = noise_chunk * std (bf16)."""
            nz = nz_tiles[b] if c < NCH - 1 else nz_tiles.pop(b)
            sc = sc_pool.tile([128, 2, I], BF16, tag="scn")
            nc.vector.tensor_tensor(sc[:], nz[:, 2 * c:2 * (c + 1), :],
                                    std_b[:, 2 * c:2 * (c + 1), :], mult)
            sc_tiles[(b, c)] = sc

        def emit_chunk_group(b, c, q, wT):
            """transpose sc chunk c half q, add mean^T -> wT o-block 2c+q."""
            ob = 2 * c + q
            sc = sc_tiles[(b, c)] if q == 0 else sc_tiles.pop((b, c))
            pt = psum_nt.tile([128, KT, 128], BF16, tag="pnt")
            for k in range(KT):
                nc.tensor.matmul(pt[:, k, :], sc[:, q, 128 * k:128 * (k + 1)],
                                 ident_b[:], is_transpose=True, start=True, stop=True)
            nc.vector.tensor_tensor(wT[:, :, 128 * ob:128 * (ob + 1)], pt[:],
                                    meanT[:, :, 128 * ob:128 * (ob + 1)], add)

        def emit_xT_group(b, m, x_nat, xT):
            """transpose x l-tile m -> xT[:, :, 128m:128(m+1)]."""
            pt = psum_xt.tile([128, KT, 128], BF16, tag="pxt")
            for k in range(KT):
                nc.tensor.matmul(pt[:, k, :], x_nat[:, m, 128 * k:128 * (k + 1)],
                                 ident_b[:], is_transpose=True, start=True, stop=True)
            nc.scalar.activation(xT[:, :, 128 * m:128 * (m + 1)], pt[:], Copy)

        def xT_slice(xT, k, m):
            return xT[:, k, 128 * m:128 * (m + 1)]

        pending_stores = {b: [] for b in range(SAMPLES)}

        def emit_store(b, m, ot):
            if m % 2 != 1:
                return
            # store l-tiles (m-1, m): 1 MB, 4 KB rows.  Stores for samples
            # 0..2 are deferred one sample so they don't steal SDMA slices
            # from the (pacing-critical) load stream mid-run.
            def fire(b=b, m=m, ot=ot):
                nc.scalar.dma_start(
                    out_d[b, 256 * (m // 2):256 * (m // 2 + 1), :]
                    .rearrange("(m p) o -> p m o", p=128),
                    ot[:, m - 1:m + 1, :])
            if b == SAMPLES - 1:
                fire()
            else:
                pending_stores[b].append(fire)

        def flush_stores(b):
            for f in pending_stores[b]:
                f()
            pending_stores[b] = []

        def emit_gemm_tile(b, m, wT, xT, ot):
            """output l-tile m, all 1024 columns: 8 shared-stationary k-steps,
            two N=512 matmuls (psum banks) per step; bias added on evac."""
            pm = psum_mm.tile([128, 2, 512], F32, tag="pmm")
            for k in range(KT):
                for n in range(2):
                    nc.tensor.matmul(pm[:, n, :], xT_slice(xT, k, m),
                                     wT[:, k, 512 * n:512 * (n + 1)],
                                     start=(k == 0), stop=(k == KT - 1))
            nc.vector.tensor_tensor(ot[:, m, :], pm[:].rearrange("p a b -> p (a b)"),
                                    bias_blk[:], add)
            emit_store(b, m, ot)

        def emit_gemm_half(b, m, n, wT, xT, ot):
            """sample-0 prologue variant: one output half (512 cols)."""
            pm = psum_mm.tile([128, 2, 512], F32, tag="pmm")
            for k in range(KT):
                nc.tensor.matmul(pm[:, 0, :], xT_slice(xT, k, m),
                                 wT[:, k, 512 * n:512 * (n + 1)],
                                 start=(k == 0), stop=(k == KT - 1))
            nc.vector.tensor_tensor(ot[:, m, 512 * n:512 * (n + 1)], pm[:, 0, :],
                                    bias_blk[:, 512 * n:512 * (n + 1)], add)
            if n == 1:
                emit_store(b, m, ot)

        # ---------------- pipeline ----------------
        def chunk_units(b, c, wT, first_mean=False):
            u = []
            if first_mean:
                u.append(lambda: (emit_exp(c), emit_mean_group(c, 0),
                                  emit_mean_group(c, 1)))
            u.append(lambda: (emit_scale(b, c), emit_chunk_group(b, c, 0, wT)))
            u.append(lambda: emit_chunk_group(b, c, 1, wT))
            return u

        def emit_loads(b):
            emit_nz_load(b, 0)
            emit_nz_load(b, 1)
            emit_x_load(b)
            emit_nz_load(b, 2)
            emit_nz_load(b, 3)

        wxT = {0: (wT_pool.tile([128, KT, O], BF16, tag="wT", name="wT0"),
                   xT_pool.tile([128, KT, L], BF16, tag="xT", name="xT0"))}
        ots = {0: out_pool.tile([128, LT, 1024], F32, tag="out", name="ot0")}

        # ---- sample-0 prologue: x^T via xbar, chunks 0,1, GEMM half 0 ----
        wT0, xT0 = wxT[0]
        x0 = x_tiles.pop(0)
        units = (chunk_units(0, 0, wT0, first_mean=True)
                 + [lambda: emit_xT_group(0, 0, x0, xT0),
                    lambda: emit_xT_group(0, 1, x0, xT0)]
                 + chunk_units(0, 1, wT0, first_mean=True)
                 + [lambda: emit_xT_group(0, 2, x0, xT0),
                    lambda: emit_xT_group(0, 3, x0, xT0)])
        for u in units:
            u()

        # bias block (via PE broadcast matmul) + warm burst right before the
        # first GEMM: real (HAM-visible) matmuls so GEMMs start at 2.4 GHz.
        for n in range(2):
            pb = psum_mm.tile([128, 2, 512], F32, tag="pmm")
            nc.tensor.matmul(pb[:, 0, :], ones_b[:], bias_b[:, 512 * n:512 * (n + 1)],
                             start=True, stop=True)
            nc.scalar.activation(bias_blk[:, 512 * n:512 * (n + 1)], pb[:, 0, :], Copy)
        pw = psum_mm.tile([128, 2, 512], F32, tag="pmm")
        for _ in range(8):
            nc.tensor.matmul(pw[:, 0, :], ident_b[:], std_b[:, 0, 0:512],
                             start=True, stop=True)

        # GEMM sample 0 half 0, interleaved with chunks 2,3 (mean slabs 2,3)
        units = chunk_units(0, 2, wT0, first_mean=True) \
            + chunk_units(0, 3, wT0, first_mean=True)
        ui = 0
        for m in range(LT):
            emit_gemm_half(0, m, 0, wT0, xT0, ots[0])
            take = 2 if m < LT - 1 else len(units) - ui
            for _ in range(take):
                if ui < len(units):
                    units[ui]()
                    ui += 1

        # ---- main loop: sample b GEMM interleaved with sample b+1 prep ----
        for b in range(SAMPLES):
            if b + 1 < SAMPLES:
                emit_loads(b + 1)
                wxT[b + 1] = (wT_pool.tile([128, KT, O], BF16, tag="wT",
                                           name=f"wT{b+1}"),
                              xT_pool.tile([128, KT, L], BF16, tag="xT",
                                           name=f"xT{b+1}"))
                ots[b + 1] = out_pool.tile([128, LT, 1024], F32, tag="out",
                                           name=f"ot{b+1}")
                wTn, xTn = wxT[b + 1]
                xn = x_tiles.pop(b + 1)
                units = []
                if b + 1 == SAMPLES - 1:
                    # last sample runs half-by-half: order its prep so
                    # chunks 0,1 + all of x^T precede chunks 2,3
                    units += chunk_units(b + 1, 0, wTn)
                    units.append(lambda: emit_xT_group(b + 1, 0, xn, xTn))
                    units += chunk_units(b + 1, 1, wTn)
                    units += [lambda m=m_: emit_xT_group(b + 1, m, xn, xTn)
                              for m_ in (1, 2, 3)]
                    units += chunk_units(b + 1, 2, wTn)
                    units += chunk_units(b + 1, 3, wTn)
                else:
                    for c in range(NCH):
                        units += chunk_units(b + 1, c, wTn)
                        units.append(lambda m=c: emit_xT_group(b + 1, m, xn, xTn))
            else:
                units = []
            wT, xT = wxT.pop(b)
            ot = ots.pop(b)
            ui = 0
            if b == SAMPLES - 1:
                # tail: half 0 needs only chunks 0,1 (its noise lands ~5.6us
                # before the second half), then half 1 closes the kernel
                for m in range(LT):
                    emit_gemm_half(b, m, 0, wT, xT, ot)
                flush_stores(b - 1)
                for m in range(LT):
                    emit_gemm_half(b, m, 1, wT, xT, ot)
            else:
                for m in range(LT):
                    if b == 0:
                        emit_gemm_half(0, m, 1, wT, xT, ot)
                    else:
                        emit_gemm_tile(b, m, wT, xT, ot)
                    take = 3 if m < LT - 1 else len(units) - ui
                    for _ in range(take):
                        if ui < len(units):
                            units[ui]()
                            ui += 1
                if b >= 1:
                    flush_stores(b - 1)

    _split_multi_waits(nc, mybir)
    return nc


def _get_nc(use_f32r=True):
    key = ("nc", use_f32r)
    if key not in _cache:
        _cache[key] = build_nc(use_f32r)
    return _cache[key]


def kernel(x, weight_mean, weight_logvar, bias, noise):
    from concourse import bass_utils

    x = np.ascontiguousarray(x, dtype=np.float32)
    noise = np.ascontiguousarray(noise, dtype=np.float32)
    weight_mean = np.ascontiguousarray(weight_mean, dtype=np.float32)
    weight_logvar = np.ascontiguousarray(weight_logvar, dtype=np.float32)
    bias = np.ascontiguousarray(bias, dtype=np.float32)

    nc = _get_nc()
    in_maps = []
    for c in range(N_CORES):
        sl = slice(SAMPLES * c, SAMPLES * (c + 1))
        in_maps.append({
            "x": x[sl], "noise": noise[sl],
            "weight_mean": weight_mean, "weight_logvar": weight_logvar,
            "bias": bias,
        })
    res = bass_utils.run_bass_kernel_spmd(nc, in_maps, list(range(N_CORES)))
    out = np.concatenate([res.results[c]["out"] for c in range(N_CORES)], axis=0)
    return out.astype(np.float32)


# revision 39
# speedup vs baseline: 1.1312x; 1.1312x over previous
"""BayesianLinear Trainium2 kernel, 8-core SPMD (data-parallel over batch).

Per-core computation (4 samples each):
    w_b = weight_mean + noise_b * exp(0.5 * weight_logvar)   (B,O,I)
    out_b = x_b @ w_b^T + bias                               (B,L,O)

Design (per core) — DMA-roofline oriented (~40 MB/core HBM traffic,
~112 us at the 358 GB/s per-core HBM limit; measured 141-155 us wall):
  - All matmul operands are bf16 (tolerance 2e-2; this path lands 2.4e-3):
    noise/x/mean are cast f32->bf16 *during* the DMA load (SWDGE cast),
    std = exp(0.5*logvar) is produced in bf16 by ACT directly (logvar
    stays f32 for exp accuracy).
  - bf16 PE transposes run at ~56 ns/block warm (FWL weight loads);
    grouped 8-deep per PSUM bank so each evac/add is one [128,8,128] op.
  - GEMM: per l-tile one 2-bank PSUM tile; each xT stationary is shared
    by two N=512 matmuls (both output halves), halving LDWEIGHTS (216 ns
    vs 259 ns per matmul).  Bias is added during the PSUM->SBUF evac
    (DVE add against a resident broadcast block) so the PE runs pure
    N=512 matmuls.
  - Software pipelining: sample b+1's transpose groups are interleaved
    between sample b's GEMM l-tiles (no >2us PE stretch without real
    matmuls, keeping the HAM clock gate at 2.4 GHz).  Samples 0 and 3
    run their GEMM half-by-half: sample 0's starts once chunks 0,1
    (~8 MB) have landed; sample 3's second half alone waits for the
    final noise bytes, shortening the tail.
  - Load order is tuned (identity mask before SWDGE descriptor
    emissions; 2 MB noise chunk-pairs; logvar on the HWDGE ring) so the
    SDMA engines stream at line rate from ~8 us (fixed preamble) on.
"""
import numpy as np

SAMPLES = 4           # batch samples per core
N_CORES = 8
B, L, I, O = 32, 512, 1024, 1024
KT = I // 128         # 8 k-tiles (contraction)
OT = O // 128         # 8 o-blocks
LT = L // 128         # 4 l-tiles
NCH = 4               # noise chunks per sample (2 o-blocks each)

_cache = {}


def _split_multi_waits(nc, mybir):
    """This walrus build allows at most one sync-wait per instruction; move
    extra waits onto preceding single-wait NOPs on the same engine.  Safe
    because kernel semaphores are monotonic between resets, so waiting
    sequentially is equivalent to waiting on the conjunction."""
    for fn in nc.m.functions:
        for bb in fn.blocks:
            insts = bb.instructions
            changed = False
            new_list = []
            for inst in insts:
                si = inst.sync_info
                if si is not None and si.on_wait and len(si.on_wait) > 1:
                    waits = list(si.on_wait)
                    for j, w in enumerate(waits[:-1]):
                        nop = mybir.InstNoOp(name=f"{inst.name}-w{j}", ins=[], outs=[])
                        nop.engine = inst.engine
                        nop.sync_info = mybir.SyncInfo(on_wait=[w], on_update=[])
                        new_list.append(nop)
                    inst.sync_info = mybir.SyncInfo(
                        on_wait=[waits[-1]], on_update=list(si.on_update or []))
                    changed = True
                new_list.append(inst)
            if changed:
                bb.instructions = new_list


def build_nc(use_f32r=True):
    from contextlib import ExitStack
    from concourse import bass, mybir, tile, masks

    F32 = mybir.dt.float32
    BF16 = mybir.dt.bfloat16
    Exp = mybir.ActivationFunctionType.Exp
    Copy = mybir.ActivationFunctionType.Copy
    mult = mybir.AluOpType.mult
    add = mybir.AluOpType.add

    nc = bass.Bass()
    x_d = nc.declare_dram_parameter("x", [SAMPLES, L, I], F32, isOutput=False)
    nz_d = nc.declare_dram_parameter("noise", [SAMPLES, O, I], F32, isOutput=False)
    wm_d = nc.declare_dram_parameter("weight_mean", [O, I], F32, isOutput=False)
    wl_d = nc.declare_dram_parameter("weight_logvar", [O, I], F32, isOutput=False)
    b_d = nc.declare_dram_parameter("bias", [O], F32, isOutput=False)
    out_d = nc.declare_dram_parameter("out", [SAMPLES, L, O], F32, isOutput=True)

    with tile.TileContext(nc) as tc, ExitStack() as ctx:
        resident = ctx.enter_context(tc.tile_pool(name="resident", bufs=1))
        lv_pool = ctx.enter_context(tc.tile_pool(name="lv", bufs=1))
        mn_pool = ctx.enter_context(tc.tile_pool(name="mn", bufs=2))
        nz_pool = ctx.enter_context(tc.tile_pool(name="nz", bufs=2))
        sc_pool = ctx.enter_context(tc.tile_pool(name="scn", bufs=2))
        xnat_pool = ctx.enter_context(tc.tile_pool(name="xnat", bufs=1))
        xT_pool = ctx.enter_context(tc.tile_pool(name="xT", bufs=2))
        wT_pool = ctx.enter_context(tc.tile_pool(name="wT", bufs=2))
        out_pool = ctx.enter_context(tc.tile_pool(name="outp", bufs=3))
        psum_mm = ctx.enter_context(tc.tile_pool(name="psum_mm", bufs=2, space="PSUM"))
        psum_nt = ctx.enter_context(tc.tile_pool(name="psum_nt", bufs=2, space="PSUM"))
        psum_xt = ctx.enter_context(tc.tile_pool(name="psum_xt", bufs=2, space="PSUM"))

        # ---------------- residents ----------------
        std_b = resident.tile([128, OT, I], BF16, tag="std")     # exp(.5 lv), natural
        meanT = resident.tile([128, KT, O], BF16, tag="meanT")   # mean^T
        ident_b = resident.tile([128, 128], BF16, tag="ident_b")
        ones_b = resident.tile([1, 128], BF16, tag="ones_b")
        bias_f = resident.tile([1, O], F32, tag="bias_f")
        bias_b = resident.tile([1, O], BF16, tag="bias_b")
        bias_blk = resident.tile([128, O], F32, tag="bias_blk")  # bias bcast to rows

        # identity first: every transpose depends on it, and it must beat the
        # SWDGE descriptor emissions into the GpSimd queue.
        masks.make_identity(nc, ident_b[:])

        # ---------------- earliest DMA issues ----------------
        # SWDGE (gpsimd) order matters: first chunk's mean+noise, then x,
        # then the rest, matching the prologue's consumption order.
        nc.sync.dma_start(bias_f[:], b_d[:].rearrange("(a o) -> a o", a=1))
        mn_tiles, nz_tiles, lv_tiles = {}, {}, {}

        def emit_lv_load(j):
            lv_tiles[j] = lv_pool.tile([128, 2, I], F32, tag="lv", name=f"lv{j}")
            nc.sync.dma_start(
                lv_tiles[j][:],
                wl_d[256 * j:256 * (j + 1), :].rearrange("(q p) i -> p q i", p=128))

        def emit_mn_load(j):
            mn_tiles[j] = mn_pool.tile([128, 2, I], BF16, tag="mn", name=f"mn{j}")
            nc.gpsimd.dma_start(
                mn_tiles[j][:],
                wm_d[256 * j:256 * (j + 1), :].rearrange("(q p) i -> p q i", p=128))

        def emit_nz_load(b, half):
            """noise for sample b: one [128, OT, I] bf16 tile, loaded in two
            2 MB SWDGE ops (chunk pairs) — semaphores stay reasonably fine-
            grained while halving ~0.8us/op descriptor-emission cost."""
            if b not in nz_tiles:
                nz_tiles[b] = nz_pool.tile([128, OT, I], BF16, tag="nz",
                                           name=f"nz{b}")
            nc.gpsimd.dma_start(
                nz_tiles[b][:, 4 * half:4 * (half + 1), :],
                nz_d[b, 512 * half:512 * (half + 1), :]
                .rearrange("(q p) i -> p q i", p=128))

        def emit_x_load(b):
            x_tiles[b] = xnat_pool.tile([128, LT, I], BF16, tag="xnat", name=f"xn{b}")
            nc.gpsimd.dma_start(
                x_tiles[b][:], x_d[b].rearrange("(m p) i -> p m i", p=128))

        x_tiles = {}
        emit_lv_load(0), emit_lv_load(1)
        emit_mn_load(0), emit_nz_load(0, 0), emit_x_load(0)
        emit_mn_load(1)
        emit_lv_load(2), emit_lv_load(3)
        emit_mn_load(2), emit_nz_load(0, 1)
        emit_mn_load(3)

        # ---------------- setup compute ----------------
        nc.vector.memset(ones_b[:], 1.0)
        nc.vector.tensor_copy(bias_b[:], bias_f[:])


        # ---------------- emitters ----------------
        def emit_mean_group(j, q):
            """transpose mean slab j, column-half q -> meanT o-block 2j+q."""
            ob = 2 * j + q
            mn = mn_tiles[j] if q == 0 else mn_tiles.pop(j)
            pt = psum_nt.tile([128, KT, 128], BF16, tag="pnt")
            for k in range(KT):
                nc.tensor.matmul(pt[:, k, :], mn[:, q, 128 * k:128 * (k + 1)],
                                 ident_b[:], is_transpose=True, start=True, stop=True)
            nc.scalar.activation(meanT[:, :, 128 * ob:128 * (ob + 1)], pt[:], Copy)

        def emit_exp(j):
            nc.scalar.activation(std_b[:, 2 * j:2 * (j + 1), :], lv_tiles.pop(j)[:],
                                 Exp, bias=0.0, scale=0.5)

        sc_tiles = {}

        def emit_scale(b, c):
            """sc = noise_chunk * std (bf16)."""
            nz = nz_tiles[b] if c < NCH - 1 else nz_tiles.pop(b)
            sc = sc_pool.tile([128, 2, I], BF16, tag="scn")
            nc.vector.tensor_tensor(sc[:], nz[:, 2 * c:2 * (c + 1), :],
                                    std_b[:, 2 * c:2 * (c + 1), :], mult)
            sc_tiles[(b, c)] = sc

        def emit_chunk_group(b, c, q, wT):
            """transpose sc chunk c half q, add mean^T -> wT o-block 2c+q."""
            ob = 2 * c + q
            sc = sc_tiles[(b, c)] if q == 0 else sc_tiles.pop((b, c))
            pt = psum_nt.tile([128, KT, 128], BF16, tag="pnt")
            for k in range(KT):
                nc.tensor.matmul(pt[:, k, :], sc[:, q, 128 * k:128 * (k + 1)],
                                 ident_b[:], is_transpose=True, start=True, stop=True)
            nc.vector.tensor_tensor(wT[:, :, 128 * ob:128 * (ob + 1)], pt[:],
                                    meanT[:, :, 128 * ob:128 * (ob + 1)], add)

        def emit_xT_group(b, m, x_nat, xT):
            """transpose x l-tile m -> xT[:, :, 128m:128(m+1)]."""
            pt = psum_xt.tile([128, KT, 128], BF16, tag="pxt")
            for k in range(KT):
                nc.tensor.matmul(pt[:, k, :], x_nat[:, m, 128 * k:128 * (k + 1)],
                                 ident_b[:], is_transpose=True, start=True, stop=True)
            nc.scalar.activation(xT[:, :, 128 * m:128 * (m + 1)], pt[:], Copy)

        def xT_slice(xT, k, m):
            return xT[:, k, 128 * m:128 * (m + 1)]

        pending_stores = {b: [] for b in range(SAMPLES)}

        def emit_store(b, m, ot):
            if m % 2 != 1:
                return
            # store l-tiles (m-1, m): 1 MB, 4 KB rows.  Stores for samples
            # 0..2 are deferred one sample so they don't steal SDMA slices
            # from the (pacing-critical) load stream mid-run.
            def fire(b=b, m=m, ot=ot):
                nc.scalar.dma_start(
                    out_d[b, 256 * (m // 2):256 * (m // 2 + 1), :]
                    .rearrange("(m p) o -> p m o", p=128),
                    ot[:, m - 1:m + 1, :])
            if b == SAMPLES - 1:
                fire()
            else:
                pending_stores[b].append(fire)

        def flush_stores(b):
            for f in pending_stores[b]:
                f()
            pending_stores[b] = []

        def emit_gemm_tile(b, m, wT, xT, ot):
            """output l-tile m, all 1024 columns: 8 shared-stationary k-steps,
            two N=512 matmuls (psum banks) per step; bias added on evac."""
            pm = psum_mm.tile([128, 2, 512], F32, tag="pmm")
            for k in range(KT):
                for n in range(2):
                    nc.tensor.matmul(pm[:, n, :], xT_slice(xT, k, m),
                                     wT[:, k, 512 * n:512 * (n + 1)],
                                     start=(k == 0), stop=(k == KT - 1))
            nc.vector.tensor_tensor(ot[:, m, :], pm[:].rearrange("p a b -> p (a b)"),
                                    bias_blk[:], add)
            emit_store(b, m, ot)

        def emit_gemm_half(b, m, n, wT, xT, ot):
            """sample-0 prologue variant: one output half (512 cols)."""
            pm = psum_mm.tile([128, 2, 512], F32, tag="pmm")
            for k in range(KT):
                nc.tensor.matmul(pm[:, 0, :], xT_slice(xT, k, m),
                                 wT[:, k, 512 * n:512 * (n + 1)],
                                 start=(k == 0), stop=(k == KT - 1))
            nc.vector.tensor_tensor(ot[:, m, 512 * n:512 * (n + 1)], pm[:, 0, :],
                                    bias_blk[:, 512 * n:512 * (n + 1)], add)
            if n == 1:
                emit_store(b, m, ot)

        # ---------------- pipeline ----------------
        def chunk_units(b, c, wT, first_mean=False):
            u = []
            if first_mean:
                u.append(lambda: (emit_exp(c), emit_mean_group(c, 0),
                                  emit_mean_group(c, 1)))
            u.append(lambda: (emit_scale(b, c), emit_chunk_group(b, c, 0, wT)))
            u.append(lambda: emit_chunk_group(b, c, 1, wT))
            return u

        def emit_loads(b):
            emit_nz_load(b, 0)
            emit_x_load(b)
            emit_nz_load(b, 1)

        wxT = {0: (wT_pool.tile([128, KT, O], BF16, tag="wT", name="wT0"),
                   xT_pool.tile([128, KT, L], BF16, tag="xT", name="xT0"))}
        ots = {0: out_pool.tile([128, LT, 1024], F32, tag="out", name="ot0")}

        # ---- sample-0 prologue: x^T via xbar, chunks 0,1, GEMM half 0 ----
        wT0, xT0 = wxT[0]
        x0 = x_tiles.pop(0)
        units = (chunk_units(0, 0, wT0, first_mean=True)
                 + [lambda: emit_xT_group(0, 0, x0, xT0),
                    lambda: emit_xT_group(0, 1, x0, xT0)]
                 + chunk_units(0, 1, wT0, first_mean=True)
                 + [lambda: emit_xT_group(0, 2, x0, xT0),
                    lambda: emit_xT_group(0, 3, x0, xT0)])
        for u in units:
            u()

        # bias block (via PE broadcast matmul) + warm burst right before the
        # first GEMM: real (HAM-visible) matmuls so GEMMs start at 2.4 GHz.
        for n in range(2):
            pb = psum_mm.tile([128, 2, 512], F32, tag="pmm")
            nc.tensor.matmul(pb[:, 0, :], ones_b[:], bias_b[:, 512 * n:512 * (n + 1)],
                             start=True, stop=True)
            nc.scalar.activation(bias_blk[:, 512 * n:512 * (n + 1)], pb[:, 0, :], Copy)
        pw = psum_mm.tile([128, 2, 512], F32, tag="pmm")
        for _ in range(8):
            nc.tensor.matmul(pw[:, 0, :], ident_b[:], std_b[:, 0, 0:512],
                             start=True, stop=True)

        # GEMM sample 0 half 0, interleaved with chunks 2,3 (mean slabs 2,3)
        units = chunk_units(0, 2, wT0, first_mean=True) \
            + chunk_units(0, 3, wT0, first_mean=True)
        ui = 0
        for m in range(LT):
            emit_gemm_half(0, m, 0, wT0, xT0, ots[0])
            take = 2 if m < LT - 1 else len(units) - ui
            for _ in range(take):
                if ui < len(units):
                    units[ui]()
                    ui += 1

        # ---- main loop: sample b GEMM interleaved with sample b+1 prep ----
        for b in range(SAMPLES):
            if b + 1 < SAMPLES:
                emit_loads(b + 1)
                wxT[b + 1] = (wT_pool.tile([128, KT, O], BF16, tag="wT",
                                           name=f"wT{b+1}"),
                              xT_pool.tile([128, KT, L], BF16, tag="xT",
                                           name=f"xT{b+1}"))
                ots[b + 1] = out_pool.tile([128, LT, 1024], F32, tag="out",
                                           name=f"ot{b+1}")
                wTn, xTn = wxT[b + 1]
                xn = x_tiles.pop(b + 1)
                units = []
                if b + 1 == SAMPLES - 1:
                    # last sample runs half-by-half: order its prep so
                    # chunks 0,1 + all of x^T precede chunks 2,3
                    units += chunk_units(b + 1, 0, wTn)
                    units.append(lambda: emit_xT_group(b + 1, 0, xn, xTn))
                    units += chunk_units(b + 1, 1, wTn)
                    units += [lambda m=m_: emit_xT_group(b + 1, m, xn, xTn)
                              for m_ in (1, 2, 3)]
                    units += chunk_units(b + 1, 2, wTn)
                    units += chunk_units(b + 1, 3, wTn)
                else:
                    for c in range(NCH):
                        units += chunk_units(b + 1, c, wTn)
                        units.append(lambda m=c: emit_xT_group(b + 1, m, xn, xTn))
            else:
                units = []
            wT, xT = wxT.pop(b)
            ot = ots.pop(b)
            ui = 0
            if b == SAMPLES - 1:
                # tail: half 0 needs only chunks 0,1 (its noise lands ~5.6us
                # before the second half), then half 1 closes the kernel
                for m in range(LT):
                    emit_gemm_half(b, m, 0, wT, xT, ot)
                flush_stores(b - 1)
                for m in range(LT):
                    emit_gemm_half(b, m, 1, wT, xT, ot)
            else:
                for m in range(LT):
                    if b == 0:
                        emit_gemm_half(0, m, 1, wT, xT, ot)
                    else:
                        emit_gemm_tile(b, m, wT, xT, ot)
                    take = 3 if m < LT - 1 else len(units) - ui
                    for _ in range(take):
                        if ui < len(units):
                            units[ui]()
                            ui += 1
                if b >= 1:
                    flush_stores(b - 1)

    _split_multi_waits(nc, mybir)
    return nc


def _get_nc(use_f32r=True):
    key = ("nc", use_f32r)
    if key not in _cache:
        _cache[key] = build_nc(use_f32r)
    return _cache[key]


def kernel(x, weight_mean, weight_logvar, bias, noise):
    from concourse import bass_utils

    x = np.ascontiguousarray(x, dtype=np.float32)
    noise = np.ascontiguousarray(noise, dtype=np.float32)
    weight_mean = np.ascontiguousarray(weight_mean, dtype=np.float32)
    weight_logvar = np.ascontiguousarray(weight_logvar, dtype=np.float32)
    bias = np.ascontiguousarray(bias, dtype=np.float32)

    nc = _get_nc()
    in_maps = []
    for c in range(N_CORES):
        sl = slice(SAMPLES * c, SAMPLES * (c + 1))
        in_maps.append({
            "x": x[sl], "noise": noise[sl],
            "weight_mean": weight_mean, "weight_logvar": weight_logvar,
            "bias": bias,
        })
    res = bass_utils.run_bass_kernel_spmd(nc, in_maps, list(range(N_CORES)))
    out = np.concatenate([res.results[c]["out"] for c in range(N_CORES)], axis=0)
    return out.astype(np.float32)
